# revision 1
# baseline (speedup 1.0000x reference)
"""CARAFE forward on 8 Trainium2 NeuronCores, data-parallel over batch.

Per core (1 sample):
  1. 1x1 conv compressor (PE, K=256 contracted in 2 chunks of 128)
  2. BatchNorm batch stats: local sum/sumsq + AllReduce over 8 cores (exact sync-BN)
  3. BN+ReLU applied in place (ACT, per-partition scale/bias)
  4. 3x3 encoder conv (PE, 9 taps PSUM-accumulated over shifted views), fused
     bias+exp on evacuation
  5. softmax over H: strided reduce over h + reciprocal + broadcast multiply
  6. reassembly: for each of 36 (tap, s) kernel maps: partition-broadcast the
     map (DMA, fp16), multiply with the shifted replicate-padded x (DVE fp16
     2x mode), accumulate the 9 taps on the PE via permutation matmuls whose
     lhsT pre-applies the faithful-to-source channel scramble
     (p' = (c%4)*32 + c//4, so each 32-partition psum block is one output
     (hb, wb) quadrant); psum rearranged on-chip to the final linear layout
     and stored with large contiguous DMAs.
"""
import numpy as np

import concourse.bass as bass
import concourse.tile as tile
from concourse import bacc, mybir
from concourse.bass_utils import run_bass_kernel_spmd
from concourse.masks import make_identity

F32 = mybir.dt.float32
BF16 = mybir.dt.float16  # 16-bit compute dtype (fp16: 11-bit mantissa)
AX = mybir.AxisListType
OP = mybir.AluOpType
AF = mybir.ActivationFunctionType

B, C, H, W = 8, 256, 64, 64
CC = 64          # compressed channels
S = 2            # scale factor
K = 3            # kernel size
E = S * S * K * K  # 36 encoder channels
EPS = 1e-5
NCORES = 8
HP, WP = H + 2, W + 2  # replicate-padded spatial dims
NPIX = H * W


def _ap(t, ap, extra_offset=0):
    return bass.AP(tensor=t.tensor, offset=t.offset + extra_offset, ap=ap)


def build():
    nc = bacc.Bacc("TRN2", target_bir_lowering=False, debug=False,
                   num_devices=NCORES)
    x_d = nc.dram_tensor("x", [C, H, W], F32, kind="ExternalInput").ap()
    w1_d = nc.dram_tensor("w1", [CC, C], F32, kind="ExternalInput").ap()
    b1_d = nc.dram_tensor("b1", [CC, 1], F32, kind="ExternalInput").ap()
    gamma_d = nc.dram_tensor("gamma", [CC, 1], F32, kind="ExternalInput").ap()
    beta_d = nc.dram_tensor("beta", [CC, 1], F32, kind="ExternalInput").ap()
    w2_d = nc.dram_tensor("w2", [E, CC * K * K], F32, kind="ExternalInput").ap()
    b2_d = nc.dram_tensor("b2", [E, 1], F32, kind="ExternalInput").ap()
    # perm[c, p'] = 1 iff c == 4*(p' % 32) + p' // 32 : PE-side partition
    # scramble so each psum block of 32 partitions maps to one (hb, wb) quadrant
    perm_d = nc.dram_tensor("perm", [128, 128], F32, kind="ExternalInput").ap()
    out_d = nc.dram_tensor("out", [C, S * H, S * W], F32, kind="ExternalOutput").ap()

    with tile.TileContext(nc) as tc:
        with (
            tc.tile_pool(name="persist", bufs=1) as persist,
            tc.tile_pool(name="small", bufs=1) as small,
            tc.tile_pool(name="dram", bufs=1, space="DRAM") as dram,
        ):
            # ---------- constants & weights ----------
            ident = persist.tile([128, 128], F32)
            make_identity(nc, ident)
            perm = persist.tile([128, 128], F32)
            nc.sync.dma_start(out=perm, in_=perm_d)

            w1_sb = small.tile([CC, C], F32)
            nc.sync.dma_start(out=w1_sb, in_=w1_d)
            w2_sb = small.tile([E, CC * K * K], F32)
            nc.sync.dma_start(out=w2_sb, in_=w2_d)
            b1_sb = small.tile([CC, 1], F32)
            nc.sync.dma_start(out=b1_sb, in_=b1_d)
            gamma_sb = small.tile([CC, 1], F32)
            nc.sync.dma_start(out=gamma_sb, in_=gamma_d)
            beta_sb = small.tile([CC, 1], F32)
            nc.sync.dma_start(out=beta_sb, in_=beta_d)
            b2_sb = small.tile([E, 1], F32)
            nc.sync.dma_start(out=b2_sb, in_=b2_d)

            # transposed weights via PE (stored bf16)
            w1T = persist.tile([128, 2, CC], BF16)  # (c_chunk 128, chunk, o)
            w2T = persist.tile([CC, K * K, E], BF16)  # (c, tap, e)
            perm_bf = persist.tile([128, 128], BF16)
            nc.scalar.copy(out=perm_bf, in_=perm)
            with tc.tile_pool(name="tp", bufs=2, space="PSUM") as tps:
                for ck in range(2):
                    pt = tps.tile([128, CC], F32, tag="w1t")
                    nc.tensor.transpose(pt, w1_sb[:, ck * 128:(ck + 1) * 128],
                                        ident[:CC, :CC])
                    nc.scalar.copy(out=w1T[:, ck, :], in_=pt)
                for t in range(K * K):
                    pt2 = tps.tile([CC, E], F32, tag="w2t")
                    # w2_sb row e holds (c, tap) flat; view tap t: (E, CC) stride K*K
                    src = _ap(w2_sb[:, :], [w2_sb[:, :].ap[0], [K * K, CC]], extra_offset=t)
                    nc.tensor.transpose(pt2, src, ident[:E, :E])
                    nc.scalar.copy(out=w2T[:, t, :], in_=pt2)

            # ---------- x with replicate padding, channel-major ----------
            # bf16 copies of padded x, pre-shifted by kj so every reassembly
            # product reads 4B-aligned rows (DVE 2x mode requirement)
            x_bf = [persist.tile([128, 2, HP, W], BF16, name=f"x_bf{j}")
                    for j in range(K)]
            with tc.tile_pool(name="xfp", bufs=1) as xfp:
                x_ext = xfp.tile([128, 2, HP, WP], F32)  # (c_part, chunk, hp, wp)
                for ck in range(2):
                    for hh in range(2):
                        nc.sync.dma_start(
                            out=x_ext[:, ck, 1 + hh * 32:1 + (hh + 1) * 32,
                                      1:W + 1],
                            in_=x_d[ck * 128:(ck + 1) * 128,
                                    hh * 32:(hh + 1) * 32, :])
                for ck in range(2):
                    # w pads (interior rows)
                    nc.vector.tensor_copy(out=x_ext[:, ck, 1:H + 1, 0:1],
                                          in_=x_ext[:, ck, 1:H + 1, 1:2])
                    nc.vector.tensor_copy(out=x_ext[:, ck, 1:H + 1, WP - 1:WP],
                                          in_=x_ext[:, ck, 1:H + 1, WP - 2:WP - 1])
                    # h pads (full padded rows, after w pads)
                    nc.vector.tensor_copy(out=x_ext[:, ck, 0:1, :],
                                          in_=x_ext[:, ck, 1:2, :])
                    nc.vector.tensor_copy(out=x_ext[:, ck, HP - 1:HP, :],
                                          in_=x_ext[:, ck, HP - 2:HP - 1, :])
                cast_engs = [nc.scalar.copy, nc.vector.tensor_copy,
                             nc.gpsimd.tensor_copy]
                for j in range(K):
                    for ck in range(2):
                        cast_engs[(j * 2 + ck) % 3](
                            out=x_bf[j][:, ck, :, :],
                            in_=x_ext[:, ck, :, j:j + W])

            # ---------- compressor: comp = w1 @ x + b1 (zero-padded buffer) ----------
            comp = persist.tile([CC, HP, WP], BF16)
            nc.vector.memset(comp[:, 0:1, :], 0.0)
            nc.vector.memset(comp[:, HP - 1:HP, :], 0.0)
            nc.vector.memset(comp[:, :, 0:1], 0.0)
            nc.vector.memset(comp[:, :, WP - 1:WP], 0.0)
            NCH = 8  # h rows per chunk
            with tc.tile_pool(name="cps", bufs=2, space="PSUM") as cps:
                for hc in range(H // NCH):
                    pc = cps.tile([CC, NCH, W], F32, tag="comp")
                    for ck in range(2):
                        nc.tensor.matmul(
                            pc, w1T[:, ck, :],
                            x_bf[1][:, ck, 1 + hc * NCH:1 + (hc + 1) * NCH, :],
                            start=(ck == 0), stop=(ck == 1))
                    nc.scalar.activation(
                        out=comp[:, 1 + hc * NCH:1 + (hc + 1) * NCH, 1:W + 1],
                        in_=pc, func=AF.Identity, bias=b1_sb, scale=1.0)

            # ---------- BN stats + AllReduce ----------
            stats = small.tile([CC, 2], F32)
            dump = small.tile([CC, NPIX], BF16)
            interior = comp[:, 1:H + 1, 1:W + 1]
            nc.scalar.activation(out=dump.rearrange("p (a b) -> p a b", a=H),
                                 in_=interior, func=AF.Identity,
                                 accum_out=stats[:, 0:1])
            nc.scalar.activation(out=dump.rearrange("p (a b) -> p a b", a=H),
                                 in_=interior, func=AF.Square,
                                 accum_out=stats[:, 1:2])
            cc_in = dram.tile([CC, 2], F32)
            cc_out = dram.tile([CC, 2], F32)
            nc.gpsimd.dma_start(out=cc_in[:], in_=stats)
            nc.gpsimd.collective_compute(
                "AllReduce", OP.add,
                replica_groups=[list(range(NCORES))],
                ins=[cc_in[:].opt()], outs=[cc_out[:].opt()])
            gstats = small.tile([CC, 2], F32)
            nc.gpsimd.dma_start(out=gstats, in_=cc_out[:])

            mu = small.tile([CC, 1], F32)
            var = small.tile([CC, 1], F32)
            scl = small.tile([CC, 1], F32)
            shf = small.tile([CC, 1], F32)
            inv_n = 1.0 / (B * NPIX)
            nc.vector.tensor_scalar_mul(out=mu, in0=gstats[:, 0:1], scalar1=inv_n)
            nc.vector.tensor_scalar_mul(out=var, in0=gstats[:, 1:2], scalar1=inv_n)
            nc.vector.tensor_tensor(out=shf, in0=mu, in1=mu, op=OP.mult)
            nc.vector.tensor_tensor(out=var, in0=var, in1=shf, op=OP.subtract)
            # scl = gamma / sqrt(var + eps); shf = beta - mu * scl
            eps_sb = small.tile([CC, 1], F32)
            nc.vector.memset(eps_sb, EPS)
            nc.scalar.activation(out=var, in_=var, func=AF.Sqrt, bias=eps_sb, scale=1.0)
            nc.vector.reciprocal(out=var, in_=var)
            nc.vector.tensor_tensor(out=scl, in0=gamma_sb, in1=var, op=OP.mult)
            nc.vector.tensor_tensor(out=shf, in0=mu, in1=scl, op=OP.mult)
            nc.vector.tensor_tensor(out=shf, in0=beta_sb, in1=shf, op=OP.subtract)
            # comp = relu(comp * scl + shf) on interior only (padding stays 0)
            nc.scalar.activation(out=interior, in_=interior, func=AF.Relu,
                                 bias=shf, scale=scl)

            # ---------- encoder conv + fused exp ----------
            eexp = persist.tile([E, H, W], F32)
            with tc.tile_pool(name="eps", bufs=2, space="PSUM") as eps_pool:
                for hc in range(H // NCH):
                    pe = eps_pool.tile([E, NCH, W], F32, tag="enc")
                    for t in range(K * K):
                        ki, kj = t // K, t % K
                        nc.tensor.matmul(
                            pe, w2T[:, t, :],
                            comp[:, hc * NCH + ki:hc * NCH + ki + NCH, kj:kj + W],
                            start=(t == 0), stop=(t == K * K - 1))
                    nc.scalar.activation(
                        out=eexp[:, hc * NCH:(hc + 1) * NCH, :], in_=pe,
                        func=AF.Exp, bias=b2_sb, scale=1.0)

            # ---------- softmax over h (axis=1 of (b, h, w, s2, k2)) ----------
            zrec = small.tile([E, W], F32)
            ee = eexp[:, :, :]
            # reduce over h (stride W) for each w: AP dims (w inner-outer swap)
            ee_wh = _ap(ee, [ee.ap[0], [1, W], [W, H]])
            nc.vector.tensor_reduce(out=zrec, in_=ee_wh, axis=AX.X, op=OP.add)
            nc.vector.reciprocal(out=zrec, in_=zrec)
            kern = persist.tile([E, H, W], F32)
            zb = zrec[:, :]
            nc.vector.tensor_tensor(
                out=kern, in0=ee,
                in1=_ap(zb, [zb.ap[0], [0, H], [1, W]]),
                op=OP.mult)

            # ---------- reassembly ----------
            # out[s, c, h, w] = sum_t kern[s*9+t, h, w] * x[c, h+ki-1, w+kj-1]
            # psum partitions pre-scrambled via perm: p' = (c%4)*32 + (c//4)%32
            kern_f = kern.rearrange("p a b -> p (a b)")
            kern_dr = dram.tile([E, H * W], BF16)
            nc.gpsimd.dma_start(out=kern_dr[:], in_=kern_f)
            HH = H // 2  # h rows per half-pass
            NOC = 512    # pixels per psum bank
            NH = NOC // W
            n_evac = 0
            HQ = HH  # h rows per group
            with (
                tc.tile_pool(name="mexp", bufs=4) as mpool,
                tc.tile_pool(name="prod", bufs=6) as ppool,
                tc.tile_pool(name="olin", bufs=1) as opool,
                tc.tile_pool(name="ops", bufs=1, space="PSUM") as ops_pool,
            ):
                NI = HQ * W // NOC  # psum chunks per ck
                for s in range(S * S):
                    for q in range(H // HQ):
                        pss = [ops_pool.tile([128, NOC], F32, tag=f"o{i}",
                                             name=f"ps_o_{s}_{q}_{i}")
                               for i in range(2 * NI)]
                        for t in range(K * K):
                            ki, kj = t // K, t % K
                            ch = s * K * K + t
                            mexp = mpool.tile([128, HQ, W], BF16)
                            src_row = kern_dr[ch:ch + 1,
                                              q * HQ * W:(q + 1) * HQ * W]
                            bcast_eng = nc.gpsimd if t % 2 == 0 else nc.sync
                            bcast_eng.dma_start(
                                out=mexp,
                                in_=_ap(src_row, [[0, 128], [1, HQ * W]]))
                            for ck in range(2):
                                prod = ppool.tile([128, HQ, W], BF16)
                                nc.vector.tensor_tensor(
                                    out=prod,
                                    in0=x_bf[kj][:, ck,
                                                 q * HQ + ki:q * HQ + ki + HQ,
                                                 :],
                                    in1=mexp, op=OP.mult)
                                prod_f = prod.rearrange("p a b -> p (a b)")
                                for i in range(NI):
                                    nc.tensor.matmul(
                                        pss[ck * NI + i], perm_bf,
                                        prod_f[:, i * NOC:(i + 1) * NOC],
                                        start=(t == 0), stop=(t == K * K - 1))
                        # rearrange psum into final linear layout on-chip, then
                        # store contiguous blocks.
                        # psum block kap=2*hb+wb (32 partitions) holds channels
                        # c%4==kap; value (p'=kap*32+chi, hl, w) belongs at
                        # out[s*64+ck*32+chi, 2*(q*HQ+i*NH+hl)+hb, wb*64+w]
                        for ck in range(2):
                            olin = opool.tile([32, 2 * HQ, S * W], F32)
                            ob = olin[:, :, :]
                            for i in range(NI):
                                for kap in range(4):
                                    hb, wb = kap // 2, kap % 2
                                    dst_view = _ap(
                                        ob, [ob.ap[0], [2 * S * W, NH], [1, W]],
                                        extra_offset=(2 * i * NH + hb) * S * W
                                        + wb * W)
                                    src_view = pss[ck * NI + i][
                                        kap * 32:(kap + 1) * 32, :]
                                    nc.scalar.copy(out=dst_view, in_=src_view)
                                    n_evac += 1
                            dst = out_d[s * 64 + ck * 32:s * 64 + ck * 32 + 32,
                                        q * 2 * HQ:(q + 1) * 2 * HQ, :]
                            nc.sync.dma_start(out=dst, in_=olin)
    nc.compile()
    return nc


_NC_CACHE = None


def _get_nc():
    global _NC_CACHE
    if _NC_CACHE is None:
        _NC_CACHE = build()
    return _NC_CACHE


def _perm_matrix():
    p = np.zeros((128, 128), dtype=np.float32)
    for pp in range(128):
        c = 4 * (pp % 32) + pp // 32
        p[c, pp] = 1.0
    return p


def _make_in_maps(inputs):
    x = np.ascontiguousarray(inputs["x"], dtype=np.float32)
    perm = _perm_matrix()
    in_maps = []
    for b in range(NCORES):
        in_maps.append({
            "x": np.ascontiguousarray(x[b]),
            "w1": np.ascontiguousarray(inputs["w1"], dtype=np.float32),
            "b1": np.ascontiguousarray(np.asarray(inputs["b1"], dtype=np.float32).reshape(CC, 1)),
            "gamma": np.ascontiguousarray(np.asarray(inputs["gamma"], dtype=np.float32).reshape(CC, 1)),
            "beta": np.ascontiguousarray(np.asarray(inputs["beta"], dtype=np.float32).reshape(CC, 1)),
            "w2": np.ascontiguousarray(np.asarray(inputs["w2"], dtype=np.float32).reshape(E, CC * K * K)),
            "b2": np.ascontiguousarray(np.asarray(inputs["b2"], dtype=np.float32).reshape(E, 1)),
            "perm": perm,
        })
    return in_maps


def kernel(x, w1, b1, gamma, beta, w2, b2, **kwargs):
    in_maps = _make_in_maps(dict(x=x, w1=w1, b1=b1, gamma=gamma, beta=beta,
                                 w2=w2, b2=b2))
    nc = _get_nc()
    res = run_bass_kernel_spmd(nc, in_maps, core_ids=list(range(NCORES)))
    return np.stack([res.results[b]["out"] for b in range(NCORES)], axis=0)



# revision 5
# speedup vs baseline: 1.4619x; 1.4619x over previous
"""CARAFE forward on 8 Trainium2 NeuronCores, data-parallel over batch.

Pixel-major reassembly design:
  - Host preps per sample: replicate-padded, transposed, kj-unrolled
    x as xun[66, 64, 3, 256] fp16 (row r, w, kj, c) plus channel-major
    xc[256, 4096] fp16 for the compressor.  Output is written in a
    device-friendly pixel-major layout and unscrambled on the host.
  - Compressor 1x1 conv (PE) -> sync-BN (AllReduce of sums) -> 3x3
    encoder conv with fused exp (PE+Act) -> softmax-over-h (DVE).
  - kern[36, 4096] is transposed per 128-pixel chunk on the PE to
    kT[128pix, 36] so reassembly products become per-partition-scalar
    multiplies (DVE tensor_scalar 4x mode / Act activation scale) --
    no partition-broadcast DMA at all.
  - Tap accumulation: identity-lhsT matmuls into PSUM (f32), one
    [128, 512] psum bank per (chunk, s-pair); straight Act copy to
    SBUF; contiguous DMA out.  A configurable subset of units instead
    accumulates fully on DVE in fp16 (scalar_tensor_tensor chains) to
    offload the PE; those go out through a separate fp16 tensor.
"""
import numpy as np

import concourse.bass as bass
import concourse.tile as tile
from concourse import bacc, mybir
from concourse.bass_utils import run_bass_kernel_spmd
from concourse.masks import make_identity

F32 = mybir.dt.float32
F16 = mybir.dt.float16
AX = mybir.AxisListType
OP = mybir.AluOpType
AF = mybir.ActivationFunctionType

B, C, H, W = 8, 256, 64, 64
CC = 64          # compressed channels
S = 2            # scale factor
K = 3            # kernel size
E = S * S * K * K  # 36 encoder channels
EPS = 1e-5
NCORES = 8
NPIX = H * W
NG = NPIX // 128          # 32 pixel chunks of 128 (2 rows each)
ROWB = W * K * C          # xun bytes-layout row: 64*3*256 elements
NCH = 8                   # h rows per conv chunk

# (g, sp) pair-units whose 9-tap accumulation runs on DVE (fp16) instead
# of the PE; their output goes to the fp16 tensor "o16".
DVE_PAIRS = frozenset((g, sp) for g in (15, 31) for sp in range(2))
# taps whose products (for PE pairs) run on the Act engine
ACT_TAPS = (0,)


def _dve_singles():
    return [(g, 2 * sp + sh)
            for (g, sp) in sorted(DVE_PAIRS) for sh in range(2)]


def _ap(t, ap, extra_offset=0):
    return bass.AP(tensor=t.tensor, offset=t.offset + extra_offset, ap=ap)


def build():
    nc = bacc.Bacc("TRN2", target_bir_lowering=False, debug=False,
                   num_devices=NCORES)
    xun_d = nc.dram_tensor("xun", [H + 2, ROWB], F16, kind="ExternalInput").ap()
    xc_d = nc.dram_tensor("xc", [C, NPIX], F16, kind="ExternalInput").ap()
    w1_d = nc.dram_tensor("w1", [CC, C], F32, kind="ExternalInput").ap()
    b1_d = nc.dram_tensor("b1", [CC, 1], F32, kind="ExternalInput").ap()
    gamma_d = nc.dram_tensor("gamma", [CC, 1], F32, kind="ExternalInput").ap()
    beta_d = nc.dram_tensor("beta", [CC, 1], F32, kind="ExternalInput").ap()
    w2_d = nc.dram_tensor("w2", [E, CC * K * K], F32, kind="ExternalInput").ap()
    b2_d = nc.dram_tensor("b2", [E, 1], F32, kind="ExternalInput").ap()
    o32_d = nc.dram_tensor("o32", [S * S, NPIX, C], F32,
                           kind="ExternalOutput").ap()
    o16_d = nc.dram_tensor("o16", [S * S, NPIX, C], F16,
                           kind="ExternalOutput").ap()

    with tile.TileContext(nc) as tc:
        with (
            tc.tile_pool(name="persist", bufs=1) as persist,
            tc.tile_pool(name="small", bufs=1) as small,
            tc.tile_pool(name="dram", bufs=1, space="DRAM") as dram,
        ):
            # ---------- constants & weights ----------
            ident = persist.tile([128, 128], F32)
            make_identity(nc, ident)
            ident16 = persist.tile([128, 128], F16)
            nc.vector.tensor_copy(out=ident16, in_=ident)

            w1_sb = small.tile([CC, C], F32)
            nc.sync.dma_start(out=w1_sb, in_=w1_d)
            w2_sb = small.tile([E, CC * K * K], F32)
            nc.sync.dma_start(out=w2_sb, in_=w2_d)
            b1_sb = small.tile([CC, 1], F32)
            nc.sync.dma_start(out=b1_sb, in_=b1_d)
            gamma_sb = small.tile([CC, 1], F32)
            nc.sync.dma_start(out=gamma_sb, in_=gamma_d)
            beta_sb = small.tile([CC, 1], F32)
            nc.sync.dma_start(out=beta_sb, in_=beta_d)
            b2_sb = small.tile([E, 1], F32)
            nc.sync.dma_start(out=b2_sb, in_=b2_d)

            # transposed weights via PE (stored fp16)
            w1T = persist.tile([128, 2, CC], F16)  # (c_part, chunk, o)
            w2T = persist.tile([CC, K * K, E], F16)  # (c, tap, e)
            with tc.tile_pool(name="tp", bufs=2, space="PSUM") as tps:
                for ck in range(2):
                    pt = tps.tile([128, CC], F32, tag="w1t")
                    nc.tensor.transpose(pt, w1_sb[:, ck * 128:(ck + 1) * 128],
                                        ident[:CC, :CC])
                    nc.scalar.copy(out=w1T[:, ck, :], in_=pt)
                for t in range(K * K):
                    pt2 = tps.tile([CC, E], F32, tag="w2t")
                    src = _ap(w2_sb[:, :], [w2_sb[:, :].ap[0], [K * K, CC]],
                              extra_offset=t)
                    nc.tensor.transpose(pt2, src, ident[:E, :E])
                    nc.scalar.copy(out=w2T[:, t, :], in_=pt2)

            # ---------- channel-major x for the compressor ----------
            xc_sb = persist.tile([128, 2, NPIX], F16)
            for ck in range(2):
                nc.sync.dma_start(out=xc_sb[:, ck, :],
                                  in_=xc_d[ck * 128:(ck + 1) * 128, :])

            # ---------- compressor: comp = w1 @ x + b1 (zero-padded) ----------
            comp = persist.tile([CC, H + 2, W + 2], F16)
            nc.vector.memset(comp[:, 0:1, :], 0.0)
            nc.vector.memset(comp[:, H + 1:H + 2, :], 0.0)
            nc.vector.memset(comp[:, :, 0:1], 0.0)
            nc.vector.memset(comp[:, :, W + 1:W + 2], 0.0)
            with tc.tile_pool(name="cps", bufs=2, space="PSUM") as cps:
                for hc in range(H // NCH):
                    pc = cps.tile([CC, NCH, W], F32, tag="comp")
                    for ck in range(2):
                        nc.tensor.matmul(
                            pc, w1T[:, ck, :],
                            _ap(xc_sb[:, ck, :],
                                [xc_sb[:, ck, :].ap[0], [W, NCH], [1, W]],
                                extra_offset=hc * NCH * W),
                            start=(ck == 0), stop=(ck == 1))
                    nc.scalar.activation(
                        out=comp[:, 1 + hc * NCH:1 + (hc + 1) * NCH, 1:W + 1],
                        in_=pc, func=AF.Identity, bias=b1_sb, scale=1.0)

            # ---------- BN stats + AllReduce ----------
            stats = small.tile([CC, 2], F32)
            dump = small.tile([CC, NPIX], F16)
            interior = comp[:, 1:H + 1, 1:W + 1]
            nc.scalar.activation(out=dump.rearrange("p (a b) -> p a b", a=H),
                                 in_=interior, func=AF.Identity,
                                 accum_out=stats[:, 0:1])
            nc.scalar.activation(out=dump.rearrange("p (a b) -> p a b", a=H),
                                 in_=interior, func=AF.Square,
                                 accum_out=stats[:, 1:2])
            cc_in = dram.tile([CC, 2], F32)
            cc_out = dram.tile([CC, 2], F32)
            nc.gpsimd.dma_start(out=cc_in[:], in_=stats)
            nc.gpsimd.collective_compute(
                "AllReduce", OP.add,
                replica_groups=[list(range(NCORES))],
                ins=[cc_in[:].opt()], outs=[cc_out[:].opt()])
            gstats = small.tile([CC, 2], F32)
            nc.gpsimd.dma_start(out=gstats, in_=cc_out[:])

            mu = small.tile([CC, 1], F32)
            var = small.tile([CC, 1], F32)
            scl = small.tile([CC, 1], F32)
            shf = small.tile([CC, 1], F32)
            inv_n = 1.0 / (B * NPIX)
            nc.vector.tensor_scalar_mul(out=mu, in0=gstats[:, 0:1], scalar1=inv_n)
            nc.vector.tensor_scalar_mul(out=var, in0=gstats[:, 1:2], scalar1=inv_n)
            nc.vector.tensor_tensor(out=shf, in0=mu, in1=mu, op=OP.mult)
            nc.vector.tensor_tensor(out=var, in0=var, in1=shf, op=OP.subtract)
            eps_sb = small.tile([CC, 1], F32)
            nc.vector.memset(eps_sb, EPS)
            nc.scalar.activation(out=var, in_=var, func=AF.Sqrt, bias=eps_sb,
                                 scale=1.0)
            nc.vector.reciprocal(out=var, in_=var)
            nc.vector.tensor_tensor(out=scl, in0=gamma_sb, in1=var, op=OP.mult)
            nc.vector.tensor_tensor(out=shf, in0=mu, in1=scl, op=OP.mult)
            nc.vector.tensor_tensor(out=shf, in0=beta_sb, in1=shf,
                                    op=OP.subtract)
            nc.scalar.activation(out=interior, in_=interior, func=AF.Relu,
                                 bias=shf, scale=scl)

            # ---------- encoder conv + fused exp ----------
            eexp = persist.tile([E, H, W], F16)
            with tc.tile_pool(name="eps", bufs=2, space="PSUM") as eps_pool:
                for hc in range(H // NCH):
                    pe = eps_pool.tile([E, NCH, W], F32, tag="enc")
                    for t in range(K * K):
                        ki, kj = t // K, t % K
                        nc.tensor.matmul(
                            pe, w2T[:, t, :],
                            comp[:, hc * NCH + ki:hc * NCH + ki + NCH,
                                 kj:kj + W],
                            start=(t == 0), stop=(t == K * K - 1))
                    nc.scalar.activation(
                        out=eexp[:, hc * NCH:(hc + 1) * NCH, :], in_=pe,
                        func=AF.Exp, bias=b2_sb, scale=1.0)

            # ---------- softmax over h ----------
            zrec = small.tile([E, W], F32)
            ee = eexp[:, :, :]
            ee_wh = _ap(ee, [ee.ap[0], [1, W], [W, H]])
            nc.vector.tensor_reduce(out=zrec, in_=ee_wh, axis=AX.X, op=OP.add)
            nc.vector.reciprocal(out=zrec, in_=zrec)
            zrec16 = small.tile([E, W], F16)
            nc.vector.tensor_copy(out=zrec16, in_=zrec)
            kern = persist.tile([E, H, W], F16)
            zb = zrec16[:, :]
            nc.vector.tensor_tensor(
                out=kern, in0=ee, in1=_ap(zb, [zb.ap[0], [0, H], [1, W]]),
                op=OP.mult)

            # ---------- kern -> pixel-major kT via PE transposes ----------
            kT = persist.tile([128, NG, E], F32)
            kern_f = kern.rearrange("p a b -> p (a b)")
            with tc.tile_pool(name="ktp", bufs=4, space="PSUM") as ktp:
                for g in range(NG):
                    pk = ktp.tile([128, E], F16, tag="kt")
                    nc.tensor.transpose(pk, kern_f[:, g * 128:(g + 1) * 128],
                                        ident16[:E, :E])
                    nc.vector.tensor_copy(out=kT[:, g, :], in_=pk)

            # ---------- reassembly ----------
            with (
                tc.tile_pool(name="xu", bufs=8) as xup,
                tc.tile_pool(name="prod", bufs=6) as prodp,
                tc.tile_pool(name="acc", bufs=4) as accp,
                tc.tile_pool(name="stage", bufs=4) as stagep,
                tc.tile_pool(name="mps", bufs=4, space="PSUM") as mps,
            ):
                for g in range(NG):
                    xu = xup.tile([128, K, K * C], F16)
                    for hh in range(2):
                        src = _ap(xun_d, [[ROWB // W, W], [ROWB, K],
                                          [1, K * C]],
                                  extra_offset=(2 * g + hh) * ROWB)
                        nc.sync.dma_start(
                            out=xu[hh * 64:(hh + 1) * 64, :, :], in_=src)
                    for sp in range(2):
                        if (g, sp) in DVE_PAIRS:
                            for sh in range(2):
                                s = 2 * sp + sh
                                acc = accp.tile([128, C], F16)
                                nc.vector.tensor_scalar_mul(
                                    out=acc, in0=xu[:, 0, 0:C],
                                    scalar1=kT[:, g, s * 9:s * 9 + 1])
                                for t in range(1, K * K):
                                    ki, kj = t // K, t % K
                                    nc.vector.scalar_tensor_tensor(
                                        out=acc,
                                        in0=xu[:, ki, kj * C:(kj + 1) * C],
                                        scalar=kT[:, g, s * 9 + t:
                                                  s * 9 + t + 1],
                                        in1=acc, op0=OP.mult, op1=OP.add)
                                nc.sync.dma_start(
                                    out=o16_d[s, g * 128:(g + 1) * 128, :],
                                    in_=acc)
                        else:
                            ps = mps.tile([128, 2 * C], F32, tag="acc")
                            for t in range(K * K):
                                ki, kj = t // K, t % K
                                prod = prodp.tile([128, 2, C], F16)
                                for sh in range(2):
                                    s = 2 * sp + sh
                                    kcol = kT[:, g, s * 9 + t:s * 9 + t + 1]
                                    if t in ACT_TAPS:
                                        nc.scalar.activation(
                                            out=prod[:, sh, :],
                                            in_=xu[:, ki, kj * C:(kj + 1) * C],
                                            func=AF.Identity, scale=kcol)
                                    else:
                                        nc.vector.tensor_scalar_mul(
                                            out=prod[:, sh, :],
                                            in0=xu[:, ki, kj * C:(kj + 1) * C],
                                            scalar1=kcol)
                                nc.tensor.matmul(
                                    ps, ident16,
                                    prod.rearrange("p a b -> p (a b)"),
                                    start=(t == 0), stop=(t == K * K - 1))
                            stg = stagep.tile([128, 2 * C], F32)
                            nc.scalar.copy(out=stg, in_=ps)
                            dst = _ap(o32_d,
                                      [[C, 128], [NPIX * C, 2], [1, C]],
                                      extra_offset=(2 * sp * NPIX + g * 128)
                                      * C)
                            nc.sync.dma_start(out=dst, in_=stg)
    nc.compile()
    return nc


_NC_CACHE = None


def _get_nc():
    global _NC_CACHE
    if _NC_CACHE is None:
        _NC_CACHE = build()
    return _NC_CACHE


def _make_in_maps(inputs):
    x = np.ascontiguousarray(np.asarray(inputs["x"], dtype=np.float32))
    w1 = np.ascontiguousarray(np.asarray(inputs["w1"], dtype=np.float32))
    b1 = np.asarray(inputs["b1"], dtype=np.float32).reshape(CC, 1)
    gamma = np.asarray(inputs["gamma"], dtype=np.float32).reshape(CC, 1)
    beta = np.asarray(inputs["beta"], dtype=np.float32).reshape(CC, 1)
    w2 = np.asarray(inputs["w2"], dtype=np.float32).reshape(E, CC * K * K)
    b2 = np.asarray(inputs["b2"], dtype=np.float32).reshape(E, 1)
    in_maps = []
    for b in range(NCORES):
        xb = x[b]                                   # (256, 64, 64)
        xp = np.pad(xb, ((0, 0), (1, 1), (1, 1)), mode="edge")
        xt = xp.transpose(1, 2, 0)                  # (66, 66, 256)
        xun = np.stack([xt[:, kj:kj + W, :] for kj in range(K)], axis=2)
        in_maps.append({
            "xun": np.ascontiguousarray(
                xun.reshape(H + 2, ROWB).astype(np.float16)),
            "xc": np.ascontiguousarray(
                xb.reshape(C, NPIX).astype(np.float16)),
            "w1": w1,
            "b1": np.ascontiguousarray(b1),
            "gamma": np.ascontiguousarray(gamma),
            "beta": np.ascontiguousarray(beta),
            "w2": np.ascontiguousarray(w2),
            "b2": np.ascontiguousarray(b2),
        })
    return in_maps


def _unscramble(o32, o16):
    """Device pixel-major (4, 4096, 256) -> reference (256, 128, 128)."""
    out = np.asarray(o32, dtype=np.float32).copy()
    o16 = np.asarray(o16, dtype=np.float32)
    for g, s in _dve_singles():
        out[s, g * 128:(g + 1) * 128, :] = o16[s, g * 128:(g + 1) * 128, :]
    arr = out.reshape(S * S, H, W, C // 4, 2, 2)    # s, h, w, c4, hb, wb
    return np.ascontiguousarray(
        arr.transpose(0, 3, 1, 4, 5, 2)).reshape(C, S * H, S * W)


def kernel(x, w1, b1, gamma, beta, w2, b2, **kwargs):
    in_maps = _make_in_maps(dict(x=x, w1=w1, b1=b1, gamma=gamma, beta=beta,
                                 w2=w2, b2=b2))
    nc = _get_nc()
    res = run_bass_kernel_spmd(nc, in_maps, core_ids=list(range(NCORES)))
    return np.stack([_unscramble(res.results[b]["o32"], res.results[b]["o16"])
                     for b in range(NCORES)], axis=0)
